# revision 5
# baseline (speedup 1.0000x reference)
"""Trainium2 Bass kernel: 16-head causal self-attention block (QKV proj ->
causal MHA -> output proj), tensor-parallel over heads across 8 NeuronCores.

Contract: kernel(**inputs) takes FULL unsharded inputs
  x      [2, 2048, 1024] f32
  w_qkv  [1024, 3072] f32, b_qkv [3072] f32
  w_proj [1024, 1024] f32, b_proj [1024] f32
and returns the FULL output [2, 2048, 1024] f32.

Sharding: head-parallel. Core c owns global heads (2c, 2c+1):
  - column-parallel QKV (each core takes its 128 q/k/v feature columns)
  - full causal attention for its 2 heads (both batches)
  - row-parallel output projection -> partial [4096, 1024] sums
  - host reduces the 8 partials and adds b_proj.

Per-core dataflow (all matmuls fp32r, feature-major activations):
  x tiles --PE transpose--> xT chunks --matmul w--> qT,kT,vT (feature-major)
  vT --PE transpose--> v natural (+ ones column for softmax denominator)
  S^T[k,q] = matmul(lhsT=kT_head, rhs=qT_head)  (2 heads row-packed, K=64)
  P^T = exp(S^T/8) via ScalarE (causal: column-restricted + triangle mask)
  y^T/Z = matmul(lhsT=v_aug, rhs=P^T) accumulated over k blocks (M=65)
  normalize: 1/Z = exp(-ln Z), broadcast via rank-1 matmul, multiply
  out = matmul(lhsT=y^T_norm, rhs=w_proj_rows)
"""

import numpy as np
from contextlib import ExitStack

import concourse.bass as bass
import concourse.tile as tile
from concourse import bacc, mybir
from concourse.bass_utils import run_bass_kernel_spmd
from concourse.masks import make_identity, make_upper_triangular

F32 = mybir.dt.float32
F32R = mybir.dt.float32r
AF = mybir.ActivationFunctionType

N_CORES = 8
B, T, E, H, D = 2, 2048, 1024, 16, 64
TOK = B * T          # 4096 tokens
P = 128              # partitions
NT = TOK // P        # 32 token tiles
SUPER = 512          # tokens per QKV super-tile
NS = TOK // SUPER    # 8 super-tiles
KCH = E // P         # 8 contraction chunks
QTL = 512            # attention q-tile width
NQT = T // QTL       # 4 q-tiles per batch
KBL = 128            # attention k-block height
VAW = 2 * (D + 1)    # v_aug columns per token tile (two heads x (64 v + 1 ones))


def r(ap):
    return ap.bitcast(F32R)


def _emit(nc, tc, ctx):
    x_h = nc.declare_dram_parameter("x", [TOK, E], F32, isOutput=False)
    wq_h = nc.declare_dram_parameter("wq", [E, P], F32, isOutput=False)
    wk_h = nc.declare_dram_parameter("wk", [E, P], F32, isOutput=False)
    wv_h = nc.declare_dram_parameter("wv", [E, P], F32, isOutput=False)
    bq_h = nc.declare_dram_parameter("bq", [P, 1], F32, isOutput=False)
    bk_h = nc.declare_dram_parameter("bk", [P, 1], F32, isOutput=False)
    bv_h = nc.declare_dram_parameter("bv", [P, 1], F32, isOutput=False)
    wp_h = nc.declare_dram_parameter("wp", [P, E], F32, isOutput=False)
    out_h = nc.declare_dram_parameter("out", [TOK, E], F32, isOutput=True)

    xr = x_h[:].rearrange("(n p) e -> n p e", p=P)      # [32, 128, 1024]
    outr = out_h[:].rearrange("(n p) e -> n p e", p=P)  # [32, 128, 1024]

    # ---------------- persistent tiles ----------------
    const = ctx.enter_context(tc.tile_pool(name="const", bufs=1))
    ident = const.tile([P, P], F32)
    make_identity(nc, ident[:])
    mask_tri = const.tile([P, P], F32)  # mask[p, f] = 1.0 iff p <= f
    make_upper_triangular(nc, mask_tri[:], val=1.0, diag=True)
    ones64f = const.tile([1, D], F32)
    nc.vector.memset(ones64f[:], 1.0)
    ones64 = const.tile([1, D], F32R)
    nc.vector.tensor_copy(ones64[:], ones64f[:])
    ones1 = const.tile([P, 1], F32)
    nc.vector.memset(ones1[:], 1.0)

    wq_sb = const.tile([P, E], F32R)
    wk_sb = const.tile([P, E], F32R)
    wv_sb = const.tile([P, E], F32R)
    wp_sb = const.tile([P, E], F32R)
    with ExitStack() as wctx:
        wstage = wctx.enter_context(tc.tile_pool(name="wstage", bufs=2))
        for wsb, wh in ((wq_sb, wq_h), (wk_sb, wk_h), (wv_sb, wv_h),
                        (wp_sb, wp_h)):
            ws = wstage.tile([P, E], F32, tag="ws", name="ws")
            if wh is wp_h:
                nc.sync.dma_start(ws[:], wh[:])
            else:
                for ch in range(KCH):
                    nc.sync.dma_start(ws[:, ch * P:(ch + 1) * P],
                                      wh[ch * P:(ch + 1) * P, :])
            nc.vector.tensor_copy(wsb[:], ws[:])
    bq_sb = const.tile([P, 1], F32)
    bk_sb = const.tile([P, 1], F32)
    bv_sb = const.tile([P, 1], F32)
    nc.sync.dma_start(bq_sb[:], bq_h[:])
    nc.sync.dma_start(bk_sb[:], bk_h[:])
    nc.sync.dma_start(bv_sb[:], bv_h[:])

    persist = ctx.enter_context(tc.tile_pool(name="persist", bufs=1))
    qT = persist.tile([P, TOK], F32R, tag="qT")    # [2 heads x 64, tokens]
    kT = persist.tile([P, TOK], F32R, tag="kT")
    yTn = persist.tile([P, TOK], F32R, tag="yTn")  # normalized y^T
    v_aug = persist.tile([P, NT * VAW], F32R, tag="vaug")
    for ti in range(NT):  # ones columns for the softmax-denominator row
        nc.vector.tensor_copy(v_aug[:, ti * VAW + D:ti * VAW + D + 1], ones1[:])
        nc.vector.tensor_copy(v_aug[:, ti * VAW + 2 * D + 1:ti * VAW + 2 * D + 2],
                              ones1[:])

    # ---------------- phase A: QKV projections ----------------
    with ExitStack() as ph:
        xpool = ph.enter_context(tc.tile_pool(name="xp", bufs=5))
        xTpool = ph.enter_context(tc.tile_pool(name="xTp", bufs=12))
        vtpool = ph.enter_context(tc.tile_pool(name="vtp", bufs=2))
        psA = ph.enter_context(tc.tile_pool(name="psA", bufs=2, space="PSUM"))
        psF = ph.enter_context(tc.tile_pool(name="psF", bufs=2, space="PSUM"))
        psV = ph.enter_context(tc.tile_pool(name="psV", bufs=2, space="PSUM"))

        for s in range(NS):
            xts = []
            for tt in range(4):
                xt = xpool.tile([P, E], F32, tag="x")
                nc.sync.dma_start(xt[:], xr[4 * s + tt])
                xts.append(xt)
            xTs = []
            for ch in range(KCH):
                ps = psA.tile([P, SUPER], F32, tag="a")
                for tt in range(4):
                    nc.tensor.transpose(
                        ps[:, tt * P:(tt + 1) * P],
                        xts[tt][:, ch * P:(ch + 1) * P],
                        ident[:],
                    )
                xTt = xTpool.tile([P, SUPER], F32R, tag="xT")
                nc.vector.tensor_copy(xTt[:], ps[:])
                xTs.append(xTt)
            # q, k: keep feature-major; v: transpose to natural + ones col
            for which, wsb, bsb in (("q", wq_sb, bq_sb), ("k", wk_sb, bk_sb),
                                    ("v", wv_sb, bv_sb)):
                pf = psF.tile([P, SUPER], F32, tag="f")
                for ch in range(KCH):
                    nc.tensor.matmul(
                        pf[:],
                        lhsT=wsb[:, ch * P:(ch + 1) * P],
                        rhs=xTs[ch][:],
                        start=(ch == 0),
                        stop=(ch == KCH - 1),
                    )
                if which == "q":
                    nc.scalar.activation(qT[:, s * SUPER:(s + 1) * SUPER], pf[:],
                                         AF.Identity, bias=bq_sb[:])
                elif which == "k":
                    nc.scalar.activation(kT[:, s * SUPER:(s + 1) * SUPER], pf[:],
                                         AF.Identity, bias=bk_sb[:])
                else:
                    vt = vtpool.tile([P, SUPER], F32, tag="vt")
                    nc.scalar.activation(vt[:], pf[:], AF.Identity, bias=bv_sb[:])
                    for tt in range(4):
                        pv = psV.tile([P, P], F32, tag="v")
                        nc.tensor.transpose(pv[:], vt[:, tt * P:(tt + 1) * P],
                                            ident[:])
                        ti = 4 * s + tt
                        nc.vector.tensor_copy(
                            v_aug[:, ti * VAW:ti * VAW + D], pv[:, 0:D])
                        nc.vector.tensor_copy(
                            v_aug[:, ti * VAW + D + 1:ti * VAW + 2 * D + 1],
                            pv[:, D:2 * D])

    # ---------------- phase B: causal attention ----------------
    with ExitStack() as ph:
        psS = ph.enter_context(tc.tile_pool(name="psS", bufs=2, space="PSUM"))
        psY = ph.enter_context(tc.tile_pool(name="psY", bufs=2, space="PSUM"))
        pTpool = ph.enter_context(tc.tile_pool(name="pTp", bufs=3))
        zpool = ph.enter_context(tc.tile_pool(name="zp", bufs=3))
        zbpool = ph.enter_context(tc.tile_pool(name="zbp", bufs=2))

        for b in range(B):
            for qi in range(NQT):
                q0 = b * T + qi * QTL  # global q column base
                nkb = 4 * qi + 4       # k blocks of 128 covering [0, (qi+1)*512)
                pys = [psY.tile([D + 1, QTL], F32, tag="y", name=f"py{h}")
                       for h in range(2)]
                for kb in range(nkb):
                    c0 = max(0, kb * KBL - qi * QTL)
                    diag = kb * KBL >= qi * QTL
                    ps = psS.tile([P, 2 * QTL], F32, tag="s")
                    for h in range(2):
                        nc.tensor.matmul(
                            ps[:, h * QTL + c0:(h + 1) * QTL],
                            lhsT=kT[64 * h:64 * h + 64,
                                    b * T + kb * KBL:b * T + (kb + 1) * KBL],
                            rhs=qT[64 * h:64 * h + 64, q0 + c0:q0 + QTL],
                            start=True, stop=True,
                        )
                    pt = pTpool.tile([P, 2 * QTL], F32R, tag="pT")
                    src = ps[:].rearrange("p (h q) -> p h q", h=2)[:, :, c0:]
                    dst = pt[:].rearrange("p (h q) -> p h q", h=2)[:, :, c0:]
                    nc.scalar.activation(dst, src, AF.Exp, scale=0.125)
                    if diag:
                        for h in range(2):
                            sl = pt[:, h * QTL + c0:h * QTL + c0 + P]
                            nc.vector.tensor_mul(sl, sl, mask_tri[:])
                    ti = b * (T // P) + kb
                    for h in range(2):
                        nc.tensor.matmul(
                            pys[h][0:D + 1, c0:QTL],
                            lhsT=v_aug[:, ti * VAW + (D + 1) * h:
                                       ti * VAW + (D + 1) * h + D + 1],
                            rhs=pt[:, h * QTL + c0:(h + 1) * QTL],
                            start=(kb == 0), stop=(kb == nkb - 1),
                        )
                # normalize: yTn = y * (1/Z), 1/Z = exp(-ln Z)
                pz = psS.tile([D, 2 * QTL], F32, tag="s")
                zrs = []
                for h in range(2):
                    zln = zpool.tile([1, QTL], F32, tag="z")
                    nc.scalar.activation(zln[:], pys[h][D:D + 1, :], AF.Ln)
                    zr = zpool.tile([1, QTL], F32R, tag="z", name="zr")
                    nc.scalar.activation(zr[:], zln[:], AF.Exp, scale=-1.0)
                    zrs.append(zr)
                    nc.tensor.matmul(
                        pz[:, h * QTL:(h + 1) * QTL],
                        lhsT=ones64[:], rhs=zr[:],
                        start=True, stop=True,
                    )
                zb = zbpool.tile([D, 2 * QTL], F32, tag="zb")
                nc.vector.tensor_copy(zb[:], pz[:])
                for h in range(2):
                    nc.vector.tensor_mul(
                        yTn[64 * h:64 * h + D, q0:q0 + QTL],
                        pys[h][0:D, :],
                        zb[:, h * QTL:(h + 1) * QTL],
                    )

    # ---------------- phase C: output projection ----------------
    with ExitStack() as ph:
        psO = ph.enter_context(tc.tile_pool(name="psO", bufs=2, space="PSUM"))
        opool = ph.enter_context(tc.tile_pool(name="op", bufs=3))
        for ti in range(NT):
            po = psO.tile([P, E], F32, tag="o")
            for oc in range(2):
                nc.tensor.matmul(
                    po[:, oc * 512:(oc + 1) * 512],
                    lhsT=yTn[:, ti * P:(ti + 1) * P],
                    rhs=wp_sb[:, oc * 512:(oc + 1) * 512],
                    start=True, stop=True,
                )
            ot = opool.tile([P, E], F32, tag="ot")
            if ti % 2 == 0:
                nc.scalar.activation(ot[:], po[:], AF.Copy)
            else:
                nc.vector.tensor_copy(ot[:], po[:])
            nc.sync.dma_start(outr[ti], ot[:])


_NC_CACHE = None


def _build():
    global _NC_CACHE
    if _NC_CACHE is None:
        nc = bacc.Bacc("TRN2", target_bir_lowering=False, debug=False)
        with tile.TileContext(nc) as tc:
            with ExitStack() as ctx:
                _emit(nc, tc, ctx)
        nc.compile()
        _NC_CACHE = nc
    return _NC_CACHE


def make_in_maps(x, w_qkv, b_qkv, w_proj):
    x2 = np.ascontiguousarray(np.asarray(x, dtype=np.float32).reshape(TOK, E))
    w_qkv = np.asarray(w_qkv, dtype=np.float32)
    b_qkv = np.asarray(b_qkv, dtype=np.float32)
    w_proj = np.asarray(w_proj, dtype=np.float32)
    in_maps = []
    for c in range(N_CORES):
        lo = P * c
        in_maps.append({
            "x": x2,
            "wq": np.ascontiguousarray(w_qkv[:, lo:lo + P]),
            "wk": np.ascontiguousarray(w_qkv[:, E + lo:E + lo + P]),
            "wv": np.ascontiguousarray(w_qkv[:, 2 * E + lo:2 * E + lo + P]),
            "bq": np.ascontiguousarray(b_qkv[lo:lo + P].reshape(P, 1)),
            "bk": np.ascontiguousarray(b_qkv[E + lo:E + lo + P].reshape(P, 1)),
            "bv": np.ascontiguousarray(b_qkv[2 * E + lo:2 * E + lo + P].reshape(P, 1)),
            "wp": np.ascontiguousarray(w_proj[lo:lo + P, :]),
        })
    return in_maps


def run_sharded(inputs, trace=False, **kw):
    nc = _build()
    in_maps = make_in_maps(inputs["x"], inputs["w_qkv"], inputs["b_qkv"],
                           inputs["w_proj"])
    res = run_bass_kernel_spmd(nc, in_maps, list(range(N_CORES)), trace=trace, **kw)
    partial = np.zeros((TOK, E), dtype=np.float32)
    for i in range(N_CORES):
        partial += res.results[i]["out"]
    out = partial + np.asarray(inputs["b_proj"], dtype=np.float32)[None, :]
    return out.reshape(B, T, E), res


def kernel(**inputs) -> np.ndarray:
    out, _ = run_sharded(inputs, trace=False)
    return out
